# Initial kernel scaffold
#
"""HEPOS multi-head attention on 8 Trainium2 NeuronCores.

Problem (full shapes): q [4,2048,1024], k/v [4,8192,1024], Wq/Wk/Wv/Wo [1024,1024],
16 heads x 64 dims, HEPOS stride 16: head h attends keys at positions h::16
(L = 512 keys/head).  Since stride == n_head, each key position feeds exactly
one head, so K/V only ever need projecting through that head's 64 columns:
the K/V projections shrink 16x vs. the dense reference.

Sharding: 8 cores = 4 batches x 2 head-groups (8 heads each).  Each core:
  QT   = Wq_g @ q[b].T                          [512(qd), 2048(tq)]
  KgT_h = Wk_h @ k[b, h::16, :].T               [64, 512] per head
  VgT_h = Wv_h @ v[b, h::16, :].T -> PE-transpose -> Vg_h [512, 64]
  ST_h = KgT_h.T-matmul -> scores.T             [512(L), 2048(tq)]
  expST = exp(ST/8); PV with ones-augmented Vg -> outT(64) + denom row
  outT_h = outT_unnorm * (1/denom)  (DMA partition-broadcast of recip)
  partial = outT.T @ Wo[:, g-cols].T            [2048, 1024]
Host: out[b] = partial[2b] + partial[2b+1] + bo.

All matmul inputs are bf16 (halves DMA, full PE rate); accumulation fp32.
"""

import numpy as np
from contextlib import ExitStack

import ml_dtypes

P = 128
BF16 = ml_dtypes.bfloat16

# full-problem constants (hardcoded per harness contract)
B, TQ_F, TK_F, D_F = 4, 2048, 8192, 1024
H_F, DH, STRIDE = 16, 64, 16
NCORES = 8
NH_LOC = H_F // 2          # 8 heads per core (2 head-groups)
L_F = TK_F // STRIDE       # 512


def build_program(D=1024, TQ=2048, L=512, NH=8, num_devices=8, reps=1):
    """Build + compile the per-core Bass program.

    Device tensors (per core):
      qt  [D, TQ]      bf16   q[b].T
      kgt [D, NH, L]   bf16   gathered k, transposed
      vgt [D, NH, L]   bf16   gathered v, transposed
      wqt/wkt/wvt [D, NH*64] bf16  (weight rows for this head-group).T
      wot [NH*64, D]   bf16   Wo[:, group cols].T
      bq/bk/bv [128, NH*64/128] f32  pair-packed per-partition biases
      out [TQ, D]      f32    partial output
    """
    import concourse.bass as bass  # noqa: F401
    import concourse.tile as tile
    from concourse import bacc, mybir
    from concourse.masks import make_identity

    bf16 = mybir.dt.bfloat16
    f32 = mybir.dt.float32

    QD = NH * DH               # this core's slice of the model dim (512)
    KD = D // P                # contraction chunks (8)
    MQ = QD // P               # qd chunks == head pairs (4)
    NP = NH // 2
    LC = L // P                # L chunks (4)
    T5 = TQ // 512             # 512-wide tq chunks (4)
    T2 = TQ // 1024            # 1024-wide tq chunks (2)
    NO = max(1, D // 512)      # output col chunks
    OW = min(512, D)           # output col chunk width
    assert QD % P == 0 and D % P == 0 and L % P == 0 and TQ % 1024 == 0
    assert MQ == NP  # head pair j <=> qd chunk j

    nc = bacc.Bacc(
        "TRN2",
        target_bir_lowering=False,
        debug=False,
        enable_asserts=False,
        num_devices=num_devices,
    )

    qt = nc.dram_tensor("qt", [D, TQ], bf16, kind="ExternalInput").ap()
    kgt = nc.dram_tensor("kgt", [D, NH, L], bf16, kind="ExternalInput").ap()
    vgt = nc.dram_tensor("vgt", [D, NH, L], bf16, kind="ExternalInput").ap()
    wqt = nc.dram_tensor("wqt", [D, QD], bf16, kind="ExternalInput").ap()
    wkt = nc.dram_tensor("wkt", [D, QD], bf16, kind="ExternalInput").ap()
    wvt = nc.dram_tensor("wvt", [D, QD], bf16, kind="ExternalInput").ap()
    wot = nc.dram_tensor("wot", [QD, D], bf16, kind="ExternalInput").ap()
    bq = nc.dram_tensor("bq", [P, MQ], f32, kind="ExternalInput").ap()
    bk = nc.dram_tensor("bk", [P, MQ], f32, kind="ExternalInput").ap()
    bv = nc.dram_tensor("bv", [P, MQ], f32, kind="ExternalInput").ap()
    out = nc.dram_tensor("out", [TQ, D], f32, kind="ExternalOutput").ap()

    qt_v = qt.rearrange("(kc p) t -> p kc t", p=P)
    kgt_v = kgt.rearrange("(kc p) h l -> p kc h l", p=P)
    vgt_v = vgt.rearrange("(kc p) h l -> p kc h l", p=P)
    wqt_v = wqt.rearrange("(kc p) m -> p kc m", p=P)
    wkt_v = wkt.rearrange("(kc p) m -> p kc m", p=P)
    wvt_v = wvt.rearrange("(kc p) m -> p kc m", p=P)
    wot_v = wot.rearrange("(j p) o -> p j o", p=P)
    out_v = out.rearrange("(mt p) o -> p mt o", p=P)

    Exp = mybir.ActivationFunctionType.Exp

    with tile.TileContext(nc) as tc, ExitStack() as ctx:
        consts = ctx.enter_context(tc.tile_pool(name="consts", bufs=1))
        persist = ctx.enter_context(tc.tile_pool(name="persist", bufs=1))
        ps_ab = ctx.enter_context(tc.tile_pool(name="ps_ab", bufs=2, space="PSUM"))
        ps_sc = ctx.enter_context(tc.tile_pool(name="ps_sc", bufs=2, space="PSUM"))
        ps_pv = ctx.enter_context(tc.tile_pool(name="ps_pv", bufs=2, space="PSUM"))

        # ---- constants ----
        ident = consts.tile([P, P], bf16)
        make_identity(nc, ident)
        wq_sb = consts.tile([P, KD, QD], bf16)
        nc.sync.dma_start(wq_sb[:], wqt_v)
        wk_sb = consts.tile([P, KD, QD], bf16)
        nc.sync.dma_start(wk_sb[:], wkt_v)
        wv_sb = consts.tile([P, KD, QD], bf16)
        nc.sync.dma_start(wv_sb[:], wvt_v)
        wo_sb = consts.tile([P, MQ, D], bf16)
        nc.sync.dma_start(wo_sb[:], wot_v)
        bq_sb = consts.tile([P, MQ], f32)
        nc.sync.dma_start(bq_sb[:], bq)
        bk_sb = consts.tile([P, MQ], f32)
        nc.sync.dma_start(bk_sb[:], bk)
        bv_sb = consts.tile([P, MQ], f32)
        nc.sync.dma_start(bv_sb[:], bv)

        # persistent intermediates
        QT = persist.tile([P, MQ, TQ], bf16)          # qd (pair-packed) x tq
        KgT = persist.tile([P, NP, L], bf16)          # e (pair-packed) x L
        # Vg: per (head, L-chunk): cols 0-63 = V values, cols 64-127 = ones.
        # The ones block makes the PV matmul emit the softmax denominator
        # replicated across psum partitions 64-127 at zero extra cycles.
        Vg = persist.tile([P, NP * 2 * LC, 2 * DH], bf16)
        nc.vector.memset(Vg[:, :, DH:], 1.0)
        outT = persist.tile([P, MQ, TQ], bf16)        # hd (pair-packed) x tq

        # ---- Phase 1: Q projection ----
        for _rep in range(reps):
            _emit_phases(
                nc, tc, ctx, mybir, bf16, ps_ab, ps_sc, ps_pv,
                qt_v, kgt_v, vgt_v, out_v,
                wq_sb, wk_sb, wv_sb, wo_sb, bq_sb, bk_sb, bv_sb, ident,
                QT, KgT, Vg, outT,
                P, KD, MQ, NP, LC, T5, T2, NO, OW, DH, L, TQ, D, _rep,
            )

    nc.compile()
    return nc


def _emit_phases(nc, tc, outer_ctx, mybir, bf16, ps_ab, ps_sc, ps_pv,
                 qt_v, kgt_v, vgt_v, out_v,
                 wq_sb, wk_sb, wv_sb, wo_sb, bq_sb, bk_sb, bv_sb, ident,
                 QT, KgT, Vg, outT,
                 P, KD, MQ, NP, LC, T5, T2, NO, OW, DH, L, TQ, D, _rep):
    Exp = mybir.ActivationFunctionType.Exp
    with ExitStack() as ctx:
        # ---- Phase 1: Q projection ----
        with tc.tile_pool(name=f"qtp{_rep}", bufs=1) as qt_pool:
            qt_sb = qt_pool.tile([P, KD, TQ], bf16)
            for kc in range(KD):
                nc.sync.dma_start(qt_sb[:, kc, :], qt_v[:, kc, :])
            for m in range(MQ):
                for t in range(T5):
                    ps = ps_ab.tile([P, 512], mybir.dt.float32, tag="ab")
                    for kc in range(KD):
                        nc.tensor.matmul(
                            ps[:],
                            wq_sb[:, kc, m * P : (m + 1) * P],
                            qt_sb[:, kc, t * 512 : (t + 1) * 512],
                            start=(kc == 0),
                            stop=(kc == KD - 1),
                        )
                    nc.vector.tensor_scalar_add(
                        QT[:, m, t * 512 : (t + 1) * 512], ps[:], bq_sb[:, m : m + 1]
                    )

        kv_pool = ctx.enter_context(tc.tile_pool(name=f"kv{_rep}", bufs=2))
        vt_pool = ctx.enter_context(tc.tile_pool(name=f"vt{_rep}", bufs=2))
        exp_pool = ctx.enter_context(tc.tile_pool(name=f"expp{_rep}", bufs=3))
        norm_pool = ctx.enter_context(tc.tile_pool(name=f"normp{_rep}", bufs=3))
        out_pool = ctx.enter_context(tc.tile_pool(name=f"outp{_rep}", bufs=3))
        MT_PER_T2 = TQ // P // T2

        def kv_proj(j):
            kg_sb = kv_pool.tile([P, KD, 2, L], bf16, tag="kg")
            nc.sync.dma_start(kg_sb[:], kgt_v[:, :, 2 * j : 2 * j + 2, :])
            vg_in = kv_pool.tile([P, KD, 2, L], bf16, tag="vg")
            nc.sync.dma_start(vg_in[:], vgt_v[:, :, 2 * j : 2 * j + 2, :])

            psk = ps_pv.tile([P, 512], mybir.dt.float32, tag="pv")
            for hh in range(2):
                for kc in range(KD):
                    nc.tensor.matmul(
                        psk[hh * DH : (hh + 1) * DH, :L],
                        wk_sb[:, kc, j * P + hh * DH : j * P + (hh + 1) * DH],
                        kg_sb[:, kc, hh, :],
                        start=(kc == 0),
                        stop=(kc == KD - 1),
                        tile_position=(0, hh * DH),
                    )
            nc.vector.tensor_scalar_add(KgT[:, j, :], psk[:, :L], bk_sb[:, j : j + 1])

            psv = ps_pv.tile([P, 512], mybir.dt.float32, tag="pv")
            for hh in range(2):
                for kc in range(KD):
                    nc.tensor.matmul(
                        psv[hh * DH : (hh + 1) * DH, :L],
                        wv_sb[:, kc, j * P + hh * DH : j * P + (hh + 1) * DH],
                        vg_in[:, kc, hh, :],
                        start=(kc == 0),
                        stop=(kc == KD - 1),
                        tile_position=(0, hh * DH),
                    )
            vgt_sb = vt_pool.tile([P, L], bf16, tag="vgt")
            nc.vector.tensor_scalar_add(vgt_sb[:], psv[:, :L], bv_sb[:, j : j + 1])
            for l in range(LC):
                pst = ps_pv.tile([P, P], bf16, tag="pv")
                nc.tensor.transpose(pst[:], vgt_sb[:, l * P : (l + 1) * P], ident)
                for hh in range(2):
                    nc.vector.tensor_copy(
                        Vg[:, (j * 2 + hh) * LC + l, 0:DH],
                        pst[:, hh * DH : (hh + 1) * DH],
                    )

        def attention(j, t2):
            for hh in range(2):
                hsl = slice(hh * DH, (hh + 1) * DH)
                expst = exp_pool.tile([P, LC, 1024], bf16, tag="expst")
                for l in range(LC):
                    pss = ps_sc.tile([P, 1024], mybir.dt.float32, tag="sc")
                    for th in range(2):
                        nc.tensor.matmul(
                            pss[:, th * 512 : (th + 1) * 512],
                            KgT[hsl, j, l * P : (l + 1) * P],
                            QT[hsl, j, t2 * 1024 + th * 512 : t2 * 1024 + (th + 1) * 512],
                            start=True,
                            stop=True,
                        )
                    nc.scalar.activation(expst[:, l, :], pss[:], Exp, scale=0.125)
                for th in range(2):
                    tsl = slice(t2 * 1024 + th * 512, t2 * 1024 + (th + 1) * 512)
                    pspv = ps_pv.tile([P, 512], mybir.dt.float32, tag="pv")
                    for l in range(LC):
                        nc.tensor.matmul(
                            pspv[:],
                            Vg[:, (j * 2 + hh) * LC + l, :],
                            expst[:, l, th * 512 : (th + 1) * 512],
                            start=(l == 0),
                            stop=(l == LC - 1),
                        )
                    recip = norm_pool.tile([DH, 512], mybir.dt.float32, tag="recip")
                    nc.vector.reciprocal(recip[:], pspv[DH : 2 * DH, :])
                    nc.vector.tensor_mul(
                        outT[hh * DH : (hh + 1) * DH, j, tsl],
                        pspv[0:DH, :],
                        recip[:],
                    )

        def out_proj(t2):
            for mt in range(t2 * MT_PER_T2, (t2 + 1) * MT_PER_T2):
                for n in range(NO):
                    pso = ps_ab.tile([P, 512], mybir.dt.float32, tag="ab")
                    for jo in range(MQ):
                        nc.tensor.matmul(
                            pso[:, :OW],
                            outT[:, jo, mt * P : (mt + 1) * P],
                            wo_sb[:, jo, n * OW : (n + 1) * OW],
                            start=(jo == 0),
                            stop=(jo == MQ - 1),
                        )
                    ob = out_pool.tile([P, OW], mybir.dt.float32, tag="ob")
                    nc.vector.tensor_copy(ob[:], pso[:, :OW])
                    nc.sync.dma_start(out_v[:, mt, n * OW : (n + 1) * OW], ob[:])

        # pairs-outer for KV proj + first TQ-half attention (early start);
        # first-half out-proj then overlaps the second-half attention.
        for j in range(NP):
            kv_proj(j)
            attention(j, 0)
        out_proj(0)
        for t2 in range(1, T2):
            for j in range(NP):
                attention(j, t2)
            out_proj(t2)


_PROG = None


def _get_program():
    global _PROG
    if _PROG is None:
        _PROG = build_program(D=D_F, TQ=TQ_F, L=L_F, NH=NH_LOC, num_devices=NCORES)
    return _PROG


def make_core_inputs(q, k, v, Wq, bq, Wk, bk, Wv, bv, Wo, bo):
    """Shard the full inputs into the 8 per-core input maps."""
    D, L, S, NH = D_F, L_F, STRIDE, NH_LOC
    QD = NH * DH
    MQ = QD // P

    q = np.ascontiguousarray(np.asarray(q, np.float32))
    k = np.ascontiguousarray(np.asarray(k, np.float32))
    v = np.ascontiguousarray(np.asarray(v, np.float32))

    qT = [np.ascontiguousarray(q[b].T).astype(BF16) for b in range(B)]
    # k[b] reshaped [L, S, D]; head h uses rows h::16 -> [:, h, :]
    kr = [k[b].reshape(L, S, D) for b in range(B)]
    vr = [v[b].reshape(L, S, D) for b in range(B)]

    WqT = np.ascontiguousarray(np.asarray(Wq, np.float32).T).astype(BF16)  # [D, D]
    WkT = np.ascontiguousarray(np.asarray(Wk, np.float32).T).astype(BF16)
    WvT = np.ascontiguousarray(np.asarray(Wv, np.float32).T).astype(BF16)
    WoT = np.ascontiguousarray(np.asarray(Wo, np.float32).T).astype(BF16)  # [D, D]
    bq = np.asarray(bq, np.float32)
    bk = np.asarray(bk, np.float32)
    bv = np.asarray(bv, np.float32)

    in_maps = []
    for c in range(NCORES):
        b, g = divmod(c, 2)
        gsl = slice(g * QD, (g + 1) * QD)
        hs0 = g * NH
        kgt = np.ascontiguousarray(
            kr[b][:, hs0 : hs0 + NH, :].transpose(2, 1, 0)
        ).astype(BF16)
        vgt = np.ascontiguousarray(
            vr[b][:, hs0 : hs0 + NH, :].transpose(2, 1, 0)
        ).astype(BF16)
        in_maps.append(
            {
                "qt": qT[b],
                "kgt": kgt,
                "vgt": vgt,
                "wqt": np.ascontiguousarray(WqT[:, gsl]),
                "wkt": np.ascontiguousarray(WkT[:, gsl]),
                "wvt": np.ascontiguousarray(WvT[:, gsl]),
                "wot": np.ascontiguousarray(WoT[gsl, :]),
                "bq": np.ascontiguousarray(bq[gsl].reshape(MQ, P).T),
                "bk": np.ascontiguousarray(bk[gsl].reshape(MQ, P).T),
                "bv": np.ascontiguousarray(bv[gsl].reshape(MQ, P).T),
            }
        )
    return in_maps


def combine_outputs(results, bo):
    bo = np.asarray(bo, np.float32)
    out = np.empty((B, TQ_F, D_F), np.float32)
    for b in range(B):
        out[b] = results[2 * b]["out"] + results[2 * b + 1]["out"] + bo
    return out


def kernel(q, k, v, Wq, bq, Wk, bk, Wv, bv, Wo, bo):
    from concourse.bass_utils import run_bass_kernel_spmd

    nc = _get_program()
    in_maps = make_core_inputs(q, k, v, Wq, bq, Wk, bk, Wv, bv, Wo, bo)
    res = run_bass_kernel_spmd(nc, in_maps, core_ids=list(range(NCORES)))
    return combine_outputs(res.results, bo)



# revision 3
# speedup vs baseline: 5.5075x; 5.5075x over previous
"""HEPOS multi-head attention on 8 Trainium2 NeuronCores.

Problem (full shapes): q [4,2048,1024], k/v [4,8192,1024], Wq/Wk/Wv/Wo [1024,1024],
16 heads x 64 dims, HEPOS stride 16: head h attends keys at positions h::16
(L = 512 keys/head).  Since stride == n_head, each key position feeds exactly
one head, so K/V only ever need projecting through that head's 64 columns:
the K/V projections shrink 16x vs. the dense reference.

Sharding: 8 cores = 4 batches x 2 head-groups (8 heads each).  Each core:
  QT   = Wq_g @ q[b].T                          [512(qd), 2048(tq)]
  KgT_h = Wk_h @ k[b, h::16, :].T               [64, 512] per head
  VgT_h = Wv_h @ v[b, h::16, :].T -> PE-transpose -> Vg_h [512, 64]
  ST_h = KgT_h.T-matmul -> scores.T             [512(L), 2048(tq)]
  expST = exp(ST/8); PV with ones-augmented Vg -> outT(64) + denom row
  outT_h = outT_unnorm * (1/denom)
  partial = outT.T @ Wo[:, g-cols].T            [2048, 1024] bf16
Host: out[b] = partial[2b] + partial[2b+1] + bo.

Schedule notes (v2):
  - DMA issue order: wq (per-m chunks) + bq, qt (per-t5 chunks), then kv
    chunk j=0, wk/wv/bk/bv, then remaining kv chunks and wo.  All on the SP
    HWDGE ring; inputs strictly precede output writes.
  - Q-proj loops t-outer so the first psum tile only needs the first qt chunk.
  - V transposes batched into one psum tile per head-pair, drained with two
    strided DVE copies instead of eight narrow ones.
  - out-proj for the first tq half is interleaved into the second-half
    attention loop to keep PE busy while Act does the exps.
  - Output is bf16 (halves the store DMA); host combine upconverts.
"""

import numpy as np
from contextlib import ExitStack

import ml_dtypes

P = 128
BF16 = ml_dtypes.bfloat16

# full-problem constants (hardcoded per harness contract)
B, TQ_F, TK_F, D_F = 4, 2048, 8192, 1024
H_F, DH, STRIDE = 16, 64, 16
NCORES = 8
NH_LOC = H_F // 2          # 8 heads per core (2 head-groups)
L_F = TK_F // STRIDE       # 512


def build_program(D=1024, TQ=2048, L=512, NH=8, num_devices=8, reps=1):
    """Build + compile the per-core Bass program.

    Device tensors (per core):
      qt  [D, TQ]      bf16   q[b].T
      kgt [D, NH, L]   bf16   gathered k, transposed
      vgt [D, NH, L]   bf16   gathered v, transposed
      wqt/wkt/wvt [D, NH*64] bf16  (weight rows for this head-group).T
      wot [NH*64, D]   bf16   Wo[:, group cols].T
      bq/bk/bv [128, NH*64/128] f32  pair-packed per-partition biases
      out [TQ, D]      bf16   partial output
    """
    import concourse.bass as bass  # noqa: F401
    import concourse.tile as tile
    from concourse import bacc, mybir
    from concourse.masks import make_identity

    bf16 = mybir.dt.bfloat16
    f32 = mybir.dt.float32

    QD = NH * DH               # this core's slice of the model dim (512)
    KD = D // P                # contraction chunks (8)
    MQ = QD // P               # qd chunks == head pairs (4)
    NP = NH // 2
    LC = L // P                # L chunks (4)
    T5 = TQ // 512             # 512-wide tq chunks (4)
    T2 = TQ // 1024            # 1024-wide tq chunks (2)
    NO = max(1, D // 512)      # output col chunks
    OW = min(512, D)           # output col chunk width
    assert QD % P == 0 and D % P == 0 and L % P == 0 and TQ % 1024 == 0
    assert MQ == NP  # head pair j <=> qd chunk j

    nc = bacc.Bacc(
        "TRN2",
        target_bir_lowering=False,
        debug=False,
        enable_asserts=False,
        num_devices=num_devices,
    )

    qt = nc.dram_tensor("qt", [D, TQ], bf16, kind="ExternalInput").ap()
    kgt = nc.dram_tensor("kgt", [D, NH, L], bf16, kind="ExternalInput").ap()
    vgt = nc.dram_tensor("vgt", [D, NH, L], bf16, kind="ExternalInput").ap()
    wqt = nc.dram_tensor("wqt", [D, QD], bf16, kind="ExternalInput").ap()
    wkt = nc.dram_tensor("wkt", [D, QD], bf16, kind="ExternalInput").ap()
    wvt = nc.dram_tensor("wvt", [D, QD], bf16, kind="ExternalInput").ap()
    wot = nc.dram_tensor("wot", [QD, D], bf16, kind="ExternalInput").ap()
    bq = nc.dram_tensor("bq", [P, MQ], f32, kind="ExternalInput").ap()
    bk = nc.dram_tensor("bk", [P, MQ], f32, kind="ExternalInput").ap()
    bv = nc.dram_tensor("bv", [P, MQ], f32, kind="ExternalInput").ap()
    out = nc.dram_tensor("out", [TQ, D], bf16, kind="ExternalOutput").ap()

    qt_v = qt.rearrange("(kc p) t -> p kc t", p=P)
    kgt_v = kgt.rearrange("(kc p) h l -> p kc h l", p=P)
    vgt_v = vgt.rearrange("(kc p) h l -> p kc h l", p=P)
    wqt_v = wqt.rearrange("(kc p) m -> p kc m", p=P)
    wkt_v = wkt.rearrange("(kc p) m -> p kc m", p=P)
    wvt_v = wvt.rearrange("(kc p) m -> p kc m", p=P)
    wot_v = wot.rearrange("(j p) o -> p j o", p=P)
    out_v = out.rearrange("(mt p) o -> p mt o", p=P)

    with tile.TileContext(nc) as tc, ExitStack() as ctx:
        consts = ctx.enter_context(tc.tile_pool(name="consts", bufs=1))
        persist = ctx.enter_context(tc.tile_pool(name="persist", bufs=1))
        ps_ab = ctx.enter_context(tc.tile_pool(name="ps_ab", bufs=2, space="PSUM"))
        ps_sc = ctx.enter_context(tc.tile_pool(name="ps_sc", bufs=2, space="PSUM"))
        ps_pv = ctx.enter_context(tc.tile_pool(name="ps_pv", bufs=2, space="PSUM"))

        # ---- constant tiles; DMA issue order matters (single SP ring) ----
        # wq + bq + qt first so Q-proj starts ASAP; kv j=0 + wk/wv next so
        # kv_proj(0) is ready right as Q-proj drains; rest pipelined.
        wq_sb = consts.tile([P, KD, QD], bf16)
        for m in range(MQ):
            nc.sync.dma_start(
                wq_sb[:, :, m * P : (m + 1) * P], wqt_v[:, :, m * P : (m + 1) * P]
            )
        bq_sb = consts.tile([P, MQ], f32)
        nc.sync.dma_start(bq_sb[:], bq)

        wk_sb = consts.tile([P, KD, QD], bf16)
        wv_sb = consts.tile([P, KD, QD], bf16)
        wo_sb = consts.tile([P, MQ, D], bf16)
        bk_sb = consts.tile([P, MQ], f32)
        bv_sb = consts.tile([P, MQ], f32)
        ident = consts.tile([P, P], bf16)

        # persistent intermediates
        QT = persist.tile([P, MQ, TQ], bf16)          # qd (pair-packed) x tq
        KgT = persist.tile([P, NP, L], bf16)          # e (pair-packed) x L
        # Vg: per (head, L-chunk): cols 0-63 = V values, cols 64-127 = ones.
        # The ones block makes the PV matmul emit the softmax denominator
        # replicated across psum partitions 64-127 at zero extra cycles.
        Vg = persist.tile([P, NP * 2 * LC, 2 * DH], bf16)
        outT = persist.tile([P, MQ, TQ], bf16)        # hd (pair-packed) x tq

        for _rep in range(reps):
            _emit_phases(
                nc, tc, mybir, make_identity, bf16, ps_ab, ps_sc, ps_pv,
                qt_v, kgt_v, vgt_v, out_v,
                wq_sb, wk_sb, wv_sb, wo_sb, bq_sb, bk_sb, bv_sb, ident,
                wkt_v, wvt_v, wot_v, bk, bv,
                QT, KgT, Vg, outT,
                P, KD, MQ, NP, LC, T5, T2, NO, OW, DH, L, TQ, D, _rep,
            )

    nc.compile()
    return nc


def _emit_phases(nc, tc, mybir, make_identity, bf16, ps_ab, ps_sc, ps_pv,
                 qt_v, kgt_v, vgt_v, out_v,
                 wq_sb, wk_sb, wv_sb, wo_sb, bq_sb, bk_sb, bv_sb, ident,
                 wkt_v, wvt_v, wot_v, bk, bv,
                 QT, KgT, Vg, outT,
                 P, KD, MQ, NP, LC, T5, T2, NO, OW, DH, L, TQ, D, _rep):
    Exp = mybir.ActivationFunctionType.Exp
    f32 = mybir.dt.float32
    with ExitStack() as ctx:
        qt_pool = ctx.enter_context(tc.tile_pool(name=f"qtp{_rep}", bufs=1))
        kv_pool = ctx.enter_context(tc.tile_pool(name=f"kv{_rep}", bufs=2))
        vt_pool = ctx.enter_context(tc.tile_pool(name=f"vt{_rep}", bufs=2))
        exp_pool = ctx.enter_context(tc.tile_pool(name=f"expp{_rep}", bufs=3))
        norm_pool = ctx.enter_context(tc.tile_pool(name=f"normp{_rep}", bufs=3))
        out_pool = ctx.enter_context(tc.tile_pool(name=f"outp{_rep}", bufs=3))

        # qt chunk DMAs, t-major: the (t, m) Q-proj tile only needs chunk t.
        qt_sb = qt_pool.tile([P, KD, TQ], bf16)
        for t in range(T5):
            nc.sync.dma_start(
                qt_sb[:, :, t * 512 : (t + 1) * 512],
                qt_v[:, :, t * 512 : (t + 1) * 512],
            )

        # prefetch kv chunk j=0, then the remaining consts, ahead of Q-proj.
        kv_tiles = {}

        def issue_kv_dma(j):
            kg = kv_pool.tile([P, KD, 2, L], bf16, tag="kg")
            nc.sync.dma_start(kg[:], kgt_v[:, :, 2 * j : 2 * j + 2, :])
            vg = kv_pool.tile([P, KD, 2, L], bf16, tag="vg")
            nc.sync.dma_start(vg[:], vgt_v[:, :, 2 * j : 2 * j + 2, :])
            kv_tiles[j] = (kg, vg)

        issue_kv_dma(0)
        nc.sync.dma_start(wk_sb[:], wkt_v)
        nc.sync.dma_start(wv_sb[:], wvt_v)
        nc.sync.dma_start(bk_sb[:], bk)
        nc.sync.dma_start(bv_sb[:], bv)
        if _rep == 0:
            make_identity(nc, ident)
            nc.gpsimd.memset(Vg[:, :, DH:], 1.0)

        # ---- Phase 1: Q projection (t-outer for early PE start) ----
        for t in range(T5):
            tsl = slice(t * 512, (t + 1) * 512)
            for m in range(MQ):
                ps = ps_ab.tile([P, 512], f32, tag="ab")
                for kc in range(KD):
                    nc.tensor.matmul(
                        ps[:],
                        wq_sb[:, kc, m * P : (m + 1) * P],
                        qt_sb[:, kc, tsl],
                        start=(kc == 0),
                        stop=(kc == KD - 1),
                    )
                nc.vector.tensor_scalar_add(QT[:, m, tsl], ps[:], bq_sb[:, m : m + 1])
            if t == 0:
                issue_kv_dma(1)
            elif t == 1:
                nc.sync.dma_start(wo_sb[:], wot_v)

        MT_PER_T2 = TQ // P // T2

        def kv_proj(j):
            kg_sb, vg_in = kv_tiles.pop(j)

            psk = ps_pv.tile([P, 512], f32, tag="pv")
            for hh in range(2):
                for kc in range(KD):
                    nc.tensor.matmul(
                        psk[hh * DH : (hh + 1) * DH, :L],
                        wk_sb[:, kc, j * P + hh * DH : j * P + (hh + 1) * DH],
                        kg_sb[:, kc, hh, :],
                        start=(kc == 0),
                        stop=(kc == KD - 1),
                        tile_position=(0, hh * DH),
                    )
            nc.vector.tensor_scalar_add(KgT[:, j, :], psk[:, :L], bk_sb[:, j : j + 1])

            psv = ps_pv.tile([P, 512], f32, tag="pv")
            for hh in range(2):
                for kc in range(KD):
                    nc.tensor.matmul(
                        psv[hh * DH : (hh + 1) * DH, :L],
                        wv_sb[:, kc, j * P + hh * DH : j * P + (hh + 1) * DH],
                        vg_in[:, kc, hh, :],
                        start=(kc == 0),
                        stop=(kc == KD - 1),
                        tile_position=(0, hh * DH),
                    )
            vgt_sb = vt_pool.tile([P, L], bf16, tag="vgt")
            nc.vector.tensor_scalar_add(vgt_sb[:], psv[:, :L], bv_sb[:, j : j + 1])
            return vgt_sb

        def v_transpose(j, vgt_sb):
            # 4 transposes into one psum tile, drained by 2 strided copies
            pst = ps_pv.tile([P, 512], bf16, tag="pv")
            for l in range(LC):
                nc.tensor.transpose(
                    pst[:, l * P : (l + 1) * P], vgt_sb[:, l * P : (l + 1) * P], ident
                )
            pst_v = pst.rearrange("p (l c) -> p l c", c=P)
            for hh in range(2):
                nc.vector.tensor_copy(
                    Vg[:, (j * 2 + hh) * LC : (j * 2 + hh) * LC + LC, 0:DH],
                    pst_v[:, :, hh * DH : (hh + 1) * DH],
                )

        def scores_half(j, t2, hh, expst):
            hsl = slice(hh * DH, (hh + 1) * DH)
            for l in range(LC):
                pss = ps_sc.tile([P, 1024], f32, tag="sc")
                for th in range(2):
                    nc.tensor.matmul(
                        pss[:, th * 512 : (th + 1) * 512],
                        KgT[hsl, j, l * P : (l + 1) * P],
                        QT[hsl, j, t2 * 1024 + th * 512 : t2 * 1024 + (th + 1) * 512],
                        start=True,
                        stop=True,
                    )
                nc.scalar.activation(expst[:, l, :], pss[:], Exp, scale=0.125)

        def pv_half(j, t2, hh, expst):
            for th in range(2):
                tsl = slice(t2 * 1024 + th * 512, t2 * 1024 + (th + 1) * 512)
                pspv = ps_pv.tile([P, 512], f32, tag="pv")
                for l in range(LC):
                    nc.tensor.matmul(
                        pspv[:],
                        Vg[:, (j * 2 + hh) * LC + l, :],
                        expst[:, l, th * 512 : (th + 1) * 512],
                        start=(l == 0),
                        stop=(l == LC - 1),
                    )
                recip = norm_pool.tile([DH, 512], f32, tag="recip")
                nc.vector.reciprocal(recip[:], pspv[DH : 2 * DH, :])
                nc.vector.tensor_mul(
                    outT[hh * DH : (hh + 1) * DH, j, tsl],
                    pspv[0:DH, :],
                    recip[:],
                )

        def attention(j, t2, vgt_sb=None):
            # hh=0 scores cover the V-bias latency before the transposes.
            expst0 = exp_pool.tile([P, LC, 1024], bf16, tag="expst")
            scores_half(j, t2, 0, expst0)
            if vgt_sb is not None:
                v_transpose(j, vgt_sb)
            expst1 = exp_pool.tile([P, LC, 1024], bf16, tag="expst")
            scores_half(j, t2, 1, expst1)
            pv_half(j, t2, 0, expst0)
            pv_half(j, t2, 1, expst1)

        def out_proj_chunk(mt):
            for n in range(NO):
                pso = ps_ab.tile([P, 512], f32, tag="ab")
                for jo in range(MQ):
                    nc.tensor.matmul(
                        pso[:, :OW],
                        outT[:, jo, mt * P : (mt + 1) * P],
                        wo_sb[:, jo, n * OW : (n + 1) * OW],
                        start=(jo == 0),
                        stop=(jo == MQ - 1),
                    )
                ob = out_pool.tile([P, OW], bf16, tag="ob")
                nc.vector.tensor_copy(ob[:], pso[:, :OW])
                nc.sync.dma_start(out_v[:, mt, n * OW : (n + 1) * OW], ob[:])

        # ---- Phase 2: per head-pair KV proj + first-half attention ----
        for j in range(NP):
            vgt_sb = kv_proj(j)
            if j + 2 < NP:
                issue_kv_dma(j + 2)
            attention(j, 0, vgt_sb)

        # ---- Phase 3: second-half attention interleaved with out-proj(0) ----
        for j in range(NP):
            attention(j, 1)
            for mt in range(2 * j, 2 * j + 2):
                out_proj_chunk(mt)

        # ---- Phase 4: out-proj for the second tq half ----
        for mt in range(MT_PER_T2, 2 * MT_PER_T2):
            out_proj_chunk(mt)


_PROG = None


def _get_program():
    global _PROG
    if _PROG is None:
        _PROG = build_program(D=D_F, TQ=TQ_F, L=L_F, NH=NH_LOC, num_devices=NCORES)
    return _PROG


def make_core_inputs(q, k, v, Wq, bq, Wk, bk, Wv, bv, Wo, bo):
    """Shard the full inputs into the 8 per-core input maps."""
    D, L, S, NH = D_F, L_F, STRIDE, NH_LOC
    QD = NH * DH
    MQ = QD // P

    q = np.ascontiguousarray(np.asarray(q, np.float32))
    k = np.ascontiguousarray(np.asarray(k, np.float32))
    v = np.ascontiguousarray(np.asarray(v, np.float32))

    qT = [np.ascontiguousarray(q[b].T).astype(BF16) for b in range(B)]
    # k[b] reshaped [L, S, D]; head h uses rows h::16 -> [:, h, :]
    # one [D, 16, L] permute per batch, shared by the two head-group cores
    kgt_full = [
        np.ascontiguousarray(k[b].reshape(L, S, D).transpose(2, 1, 0)).astype(BF16)
        for b in range(B)
    ]
    vgt_full = [
        np.ascontiguousarray(v[b].reshape(L, S, D).transpose(2, 1, 0)).astype(BF16)
        for b in range(B)
    ]

    WqT = np.ascontiguousarray(np.asarray(Wq, np.float32).T).astype(BF16)  # [D, D]
    WkT = np.ascontiguousarray(np.asarray(Wk, np.float32).T).astype(BF16)
    WvT = np.ascontiguousarray(np.asarray(Wv, np.float32).T).astype(BF16)
    WoT = np.ascontiguousarray(np.asarray(Wo, np.float32).T).astype(BF16)  # [D, D]
    bq = np.asarray(bq, np.float32)
    bk = np.asarray(bk, np.float32)
    bv = np.asarray(bv, np.float32)

    in_maps = []
    for c in range(NCORES):
        b, g = divmod(c, 2)
        gsl = slice(g * QD, (g + 1) * QD)
        hs0 = g * NH
        kgt = np.ascontiguousarray(kgt_full[b][:, hs0 : hs0 + NH, :])
        vgt = np.ascontiguousarray(vgt_full[b][:, hs0 : hs0 + NH, :])
        in_maps.append(
            {
                "qt": qT[b],
                "kgt": kgt,
                "vgt": vgt,
                "wqt": np.ascontiguousarray(WqT[:, gsl]),
                "wkt": np.ascontiguousarray(WkT[:, gsl]),
                "wvt": np.ascontiguousarray(WvT[:, gsl]),
                "wot": np.ascontiguousarray(WoT[gsl, :]),
                "bq": np.ascontiguousarray(bq[gsl].reshape(MQ, P).T),
                "bk": np.ascontiguousarray(bk[gsl].reshape(MQ, P).T),
                "bv": np.ascontiguousarray(bv[gsl].reshape(MQ, P).T),
            }
        )
    return in_maps


def _bf16_to_f32(a):
    return (
        a.view(np.uint16).astype(np.uint32) << np.uint32(16)
    ).view(np.float32)


def combine_outputs(results, bo):
    bo = np.asarray(bo, np.float32)
    out = np.empty((B, TQ_F, D_F), np.float32)
    for b in range(B):
        out[b] = _bf16_to_f32(results[2 * b]["out"])
        out[b] += _bf16_to_f32(results[2 * b + 1]["out"])
        out[b] += bo
    return out


def kernel(q, k, v, Wq, bq, Wk, bk, Wv, bv, Wo, bo):
    from concourse.bass_utils import run_bass_kernel_spmd

    nc = _get_program()
    in_maps = make_core_inputs(q, k, v, Wq, bq, Wk, bk, Wv, bv, Wo, bo)
    res = run_bass_kernel_spmd(nc, in_maps, core_ids=list(range(NCORES)))
    return combine_outputs(res.results, bo)


# revision 15
# speedup vs baseline: 6.3751x; 1.1575x over previous
"""HEPOS multi-head attention on 8 Trainium2 NeuronCores.

Problem (full shapes): q [4,2048,1024], k/v [4,8192,1024], Wq/Wk/Wv/Wo [1024,1024],
16 heads x 64 dims, HEPOS stride 16: head h attends keys at positions h::16
(L = 512 keys/head).  Since stride == n_head, each key position feeds exactly
one head, so K/V only ever need projecting through that head's 64 columns:
the K/V projections shrink 16x vs. the dense reference.

Sharding: 8 cores = 4 batches x 2 head-groups (8 heads each).  Each core:
  QT   = Wq_g @ q[b].T                          [512(qd), 2048(tq)]
  KgT_h = Wk_h @ k[b, h::16, :].T               [64, 512] per head
  VgT_h = Wv_h @ v[b, h::16, :].T -> PE-transpose -> Vg_h [512, 64]
  ST_h = KgT_h.T-matmul -> scores.T             [512(L), 2048(tq)]
  expST = exp(ST/8); PV with ones-augmented Vg -> outT(64) + denom row
  outT_h = outT_unnorm * (1/denom)
  partial = outT.T @ Wo[:, g-cols].T            [2048, 1024] bf16
Host: out[b] = partial[2b] + partial[2b+1] + bo.

Schedule notes (v2):
  - DMA issue order: wq (per-m chunks) + bq, qt (per-t5 chunks), then kv
    chunk j=0, wk/wv/bk/bv, then remaining kv chunks and wo.  All on the SP
    HWDGE ring; inputs strictly precede output writes.
  - Q-proj loops t-outer so the first psum tile only needs the first qt chunk.
  - V transposes batched into one psum tile per head-pair, drained with two
    strided DVE copies instead of eight narrow ones.
  - out-proj for the first tq half is interleaved into the second-half
    attention loop to keep PE busy while Act does the exps.
  - Output is bf16 (halves the store DMA); host combine upconverts.
"""

import numpy as np
from contextlib import ExitStack

import ml_dtypes

P = 128
BF16 = ml_dtypes.bfloat16

# full-problem constants (hardcoded per harness contract)
B, TQ_F, TK_F, D_F = 4, 2048, 8192, 1024
H_F, DH, STRIDE = 16, 64, 16
NCORES = 8
NH_LOC = H_F // 2          # 8 heads per core (2 head-groups)
L_F = TK_F // STRIDE       # 512


def build_program(D=1024, TQ=2048, L=512, NH=8, num_devices=8, reps=1):
    """Build + compile the per-core Bass program.

    Device tensors (per core):
      qt  [D, TQ]      bf16   q[b].T
      kgt [D, NH, L]   bf16   gathered k, transposed
      vgt [D, NH, L]   bf16   gathered v, transposed
      wqt/wkt/wvt [D, NH*64] bf16  (weight rows for this head-group).T
      wot [NH*64, D]   bf16   Wo[:, group cols].T
      bq/bk/bv [128, NH*64/128] f32  pair-packed per-partition biases
      out [TQ, D]      bf16   partial output
    """
    import concourse.bass as bass  # noqa: F401
    import concourse.tile as tile
    from concourse import bacc, mybir
    from concourse.masks import make_identity

    bf16 = mybir.dt.bfloat16
    f32 = mybir.dt.float32

    QD = NH * DH               # this core's slice of the model dim (512)
    KD = D // P                # contraction chunks (8)
    MQ = QD // P               # qd chunks == head pairs (4)
    NP = NH // 2
    LC = L // P                # L chunks (4)
    T5 = TQ // 512             # 512-wide tq chunks (4)
    T2 = TQ // 1024            # 1024-wide tq chunks (2)
    NO = max(1, D // 512)      # output col chunks
    OW = min(512, D)           # output col chunk width
    assert QD % P == 0 and D % P == 0 and L % P == 0 and TQ % 1024 == 0
    assert MQ == NP  # head pair j <=> qd chunk j

    nc = bacc.Bacc(
        "TRN2",
        target_bir_lowering=False,
        debug=False,
        enable_asserts=False,
        num_devices=num_devices,
    )

    qt = nc.dram_tensor("qt", [D, TQ], bf16, kind="ExternalInput").ap()
    kgt = nc.dram_tensor("kgt", [D, NH, L], bf16, kind="ExternalInput").ap()
    vgt = nc.dram_tensor("vgt", [D, NH, L], bf16, kind="ExternalInput").ap()
    wqt = nc.dram_tensor("wqt", [D, QD], bf16, kind="ExternalInput").ap()
    wkt = nc.dram_tensor("wkt", [D, QD], bf16, kind="ExternalInput").ap()
    wvt = nc.dram_tensor("wvt", [D, QD], bf16, kind="ExternalInput").ap()
    wot = nc.dram_tensor("wot", [QD, D], bf16, kind="ExternalInput").ap()
    bq = nc.dram_tensor("bq", [P, MQ], f32, kind="ExternalInput").ap()
    bk = nc.dram_tensor("bk", [P, MQ], f32, kind="ExternalInput").ap()
    bv = nc.dram_tensor("bv", [P, MQ], f32, kind="ExternalInput").ap()
    out = nc.dram_tensor("out", [TQ, D], bf16, kind="ExternalOutput").ap()

    qt_v = qt.rearrange("(kc p) t -> p kc t", p=P)
    kgt_v = kgt.rearrange("(kc p) h l -> p kc h l", p=P)
    vgt_v = vgt.rearrange("(kc p) h l -> p kc h l", p=P)
    wqt_v = wqt.rearrange("(kc p) m -> p kc m", p=P)
    wkt_v = wkt.rearrange("(kc p) m -> p kc m", p=P)
    wvt_v = wvt.rearrange("(kc p) m -> p kc m", p=P)
    wot_v = wot.rearrange("(j p) o -> p j o", p=P)
    out_v = out.rearrange("(mt p) o -> p mt o", p=P)

    with tile.TileContext(nc) as tc, ExitStack() as ctx:
        consts = ctx.enter_context(tc.tile_pool(name="consts", bufs=1))
        persist = ctx.enter_context(tc.tile_pool(name="persist", bufs=1))
        ps_ab = ctx.enter_context(tc.tile_pool(name="ps_ab", bufs=2, space="PSUM"))
        ps_sc = ctx.enter_context(tc.tile_pool(name="ps_sc", bufs=2, space="PSUM"))
        ps_pv = ctx.enter_context(tc.tile_pool(name="ps_pv", bufs=2, space="PSUM"))

        # ---- constant tiles; DMA issue order matters (single SP ring) ----
        # wq + bq + qt first so Q-proj starts ASAP; kv j=0 + wk/wv next so
        # kv_proj(0) is ready right as Q-proj drains; rest pipelined.
        wq_sb = consts.tile([P, KD, QD], bf16)
        nc.sync.dma_start(wq_sb[:, :, 0:P], wqt_v[:, :, 0:P])
        bq_sb = consts.tile([P, MQ], f32)

        wk_sb = consts.tile([P, KD, QD], bf16)
        wv_sb = consts.tile([P, KD, QD], bf16)
        wo_sb = consts.tile([P, MQ, D], bf16)
        bk_sb = consts.tile([P, MQ], f32)
        bv_sb = consts.tile([P, MQ], f32)
        ident = consts.tile([P, P], bf16)

        # persistent intermediates
        QT = persist.tile([P, MQ, TQ], bf16)          # qd (pair-packed) x tq
        KgT = persist.tile([P, NP, L], bf16)          # e (pair-packed) x L
        # Vg: per (head, L-chunk): cols 0-63 = V values, cols 64-127 = ones.
        # The ones block makes the PV matmul emit the softmax denominator
        # replicated across psum partitions 64-127 at zero extra cycles.
        Vg = persist.tile([P, NP * 2 * LC, 2 * DH], bf16)
        outT = persist.tile([P, MQ, TQ], bf16)        # hd (pair-packed) x tq

        for _rep in range(reps):
            _emit_phases(
                nc, tc, mybir, make_identity, bf16, ps_ab, ps_sc, ps_pv,
                qt_v, kgt_v, vgt_v, out_v,
                wq_sb, wk_sb, wv_sb, wo_sb, bq_sb, bk_sb, bv_sb, ident,
                wqt_v, wkt_v, wvt_v, wot_v, bq, bk, bv,
                QT, KgT, Vg, outT,
                P, KD, MQ, NP, LC, T5, T2, NO, OW, DH, L, TQ, D, _rep,
            )

    nc.compile()
    return nc


def _emit_phases(nc, tc, mybir, make_identity, bf16, ps_ab, ps_sc, ps_pv,
                 qt_v, kgt_v, vgt_v, out_v,
                 wq_sb, wk_sb, wv_sb, wo_sb, bq_sb, bk_sb, bv_sb, ident,
                 wqt_v, wkt_v, wvt_v, wot_v, bq, bk, bv,
                 QT, KgT, Vg, outT,
                 P, KD, MQ, NP, LC, T5, T2, NO, OW, DH, L, TQ, D, _rep):
    Exp = mybir.ActivationFunctionType.Exp
    f32 = mybir.dt.float32
    with ExitStack() as ctx:
        qt_pool = ctx.enter_context(tc.tile_pool(name=f"qtp{_rep}", bufs=1))
        kv_pool = ctx.enter_context(tc.tile_pool(name=f"kv{_rep}", bufs=2))
        vt_pool = ctx.enter_context(tc.tile_pool(name=f"vt{_rep}", bufs=2))
        exp_pool = ctx.enter_context(tc.tile_pool(name=f"expp{_rep}", bufs=2))
        norm_pool = ctx.enter_context(tc.tile_pool(name=f"normp{_rep}", bufs=2))
        out_pool = ctx.enter_context(tc.tile_pool(name=f"outp{_rep}", bufs=2))

        # qt chunk DMAs, t-major: the (t, m) Q-proj tile only needs chunk t.
        # Order: qt t=0 right after wq m=0 so the first psum tile starts ~5us
        # in; the remaining wq chunks hide behind the t=0 matmuls.
        qt_sb = qt_pool.tile([P, KD, TQ], bf16)
        nc.sync.dma_start(qt_sb[:, :, 0:512], qt_v[:, :, 0:512])
        for m in range(1, MQ):
            nc.sync.dma_start(
                wq_sb[:, :, m * P : (m + 1) * P], wqt_v[:, :, m * P : (m + 1) * P]
            )
        nc.sync.dma_start(bq_sb[:], bq)
        for t in range(1, T5):
            nc.sync.dma_start(
                qt_sb[:, :, t * 512 : (t + 1) * 512],
                qt_v[:, :, t * 512 : (t + 1) * 512],
            )

        # prefetch kv chunk j=0, then the remaining consts, ahead of Q-proj.
        kv_tiles = {}

        def issue_kv_dma(j):
            kg = kv_pool.tile([P, KD, 2, L], bf16, tag="kg")
            nc.sync.dma_start(kg[:], kgt_v[:, :, 2 * j : 2 * j + 2, :])
            vg = kv_pool.tile([P, KD, 2, L], bf16, tag="vg")
            nc.sync.dma_start(vg[:], vgt_v[:, :, 2 * j : 2 * j + 2, :])
            kv_tiles[j] = (kg, vg)

        issue_kv_dma(0)
        nc.sync.dma_start(wk_sb[:], wkt_v)
        nc.sync.dma_start(wv_sb[:], wvt_v)
        nc.sync.dma_start(bk_sb[:], bk)
        nc.sync.dma_start(bv_sb[:], bv)
        if _rep == 0:
            make_identity(nc, ident)
            nc.gpsimd.memset(Vg[:, :, DH:], 1.0)

        # ---- Phase 1: Q projection (t-outer for early PE start) ----
        for t in range(T5):
            tsl = slice(t * 512, (t + 1) * 512)
            for m in range(MQ):
                ps = ps_ab.tile([P, 512], f32, tag="ab")
                for kc in range(KD):
                    nc.tensor.matmul(
                        ps[:],
                        wq_sb[:, kc, m * P : (m + 1) * P],
                        qt_sb[:, kc, tsl],
                        start=(kc == 0),
                        stop=(kc == KD - 1),
                    )
                nc.vector.tensor_scalar_add(QT[:, m, tsl], ps[:], bq_sb[:, m : m + 1])
            if t == 0:
                issue_kv_dma(1)
            elif t == 1:
                nc.sync.dma_start(wo_sb[:], wot_v)

        MT_PER_T2 = TQ // P // T2

        def kv_proj(j):
            kg_sb, vg_in = kv_tiles.pop(j)

            psk = ps_ab.tile([P, 512], f32, tag="ab")
            for hh in range(2):
                for kc in range(KD):
                    nc.tensor.matmul(
                        psk[hh * DH : (hh + 1) * DH, :L],
                        wk_sb[:, kc, j * P + hh * DH : j * P + (hh + 1) * DH],
                        kg_sb[:, kc, hh, :],
                        start=(kc == 0),
                        stop=(kc == KD - 1),
                        tile_position=(0, hh * DH),
                    )
            nc.vector.tensor_scalar_add(KgT[:, j, :], psk[:, :L], bk_sb[:, j : j + 1])

            psv = ps_ab.tile([P, 512], f32, tag="ab")
            for hh in range(2):
                for kc in range(KD):
                    nc.tensor.matmul(
                        psv[hh * DH : (hh + 1) * DH, :L],
                        wv_sb[:, kc, j * P + hh * DH : j * P + (hh + 1) * DH],
                        vg_in[:, kc, hh, :],
                        start=(kc == 0),
                        stop=(kc == KD - 1),
                        tile_position=(0, hh * DH),
                    )
            vgt_sb = vt_pool.tile([P, L], bf16, tag="vgt")
            nc.vector.tensor_scalar_add(vgt_sb[:], psv[:, :L], bv_sb[:, j : j + 1])
            return vgt_sb

        def v_transpose(j, vgt_sb):
            # 4 transposes into one psum tile, drained by 2 strided copies
            pst = ps_ab.tile([P, 512], bf16, tag="ab")
            for l in range(LC):
                nc.tensor.transpose(
                    pst[:, l * P : (l + 1) * P], vgt_sb[:, l * P : (l + 1) * P], ident
                )
            pst_v = pst.rearrange("p (l c) -> p l c", c=P)
            for hh in range(2):
                nc.vector.tensor_copy(
                    Vg[:, (j * 2 + hh) * LC : (j * 2 + hh) * LC + LC, 0:DH],
                    pst_v[:, :, hh * DH : (hh + 1) * DH],
                )

        def scores_half(j, t2, hh, expst):
            hsl = slice(hh * DH, (hh + 1) * DH)
            for l in range(LC):
                pss = ps_sc.tile([P, 1024], f32, tag="sc")
                for th in range(2):
                    nc.tensor.matmul(
                        pss[:, th * 512 : (th + 1) * 512],
                        KgT[hsl, j, l * P : (l + 1) * P],
                        QT[hsl, j, t2 * 1024 + th * 512 : t2 * 1024 + (th + 1) * 512],
                        start=True,
                        stop=True,
                    )
                nc.scalar.activation(expst[:, l, :], pss[:], Exp, scale=0.125)

        def pv_half(j, t2, hh, expst):
            for th in range(2):
                tsl = slice(t2 * 1024 + th * 512, t2 * 1024 + (th + 1) * 512)
                pspv = ps_pv.tile([P, 512], f32, tag="pv")
                for l in range(LC):
                    nc.tensor.matmul(
                        pspv[:],
                        Vg[:, (j * 2 + hh) * LC + l, :],
                        expst[:, l, th * 512 : (th + 1) * 512],
                        start=(l == 0),
                        stop=(l == LC - 1),
                    )
                recip = norm_pool.tile([DH, 512], f32, tag="recip")
                nc.vector.reciprocal(recip[:], pspv[DH : 2 * DH, :])
                nc.vector.tensor_mul(
                    outT[hh * DH : (hh + 1) * DH, j, tsl],
                    pspv[0:DH, :],
                    recip[:],
                )

        def attention(j, t2, vgt_sb=None):
            # hh=0 scores cover the V-bias latency before the transposes.
            expst0 = exp_pool.tile([P, LC, 1024], bf16, tag="expst")
            scores_half(j, t2, 0, expst0)
            if vgt_sb is not None:
                v_transpose(j, vgt_sb)
            expst1 = exp_pool.tile([P, LC, 1024], bf16, tag="expst")
            scores_half(j, t2, 1, expst1)
            pv_half(j, t2, 0, expst0)
            pv_half(j, t2, 1, expst1)

        def out_proj_chunk(mt):
            for n in range(NO):
                pso = ps_ab.tile([P, 512], f32, tag="ab")
                for jo in range(MQ):
                    nc.tensor.matmul(
                        pso[:, :OW],
                        outT[:, jo, mt * P : (mt + 1) * P],
                        wo_sb[:, jo, n * OW : (n + 1) * OW],
                        start=(jo == 0),
                        stop=(jo == MQ - 1),
                    )
                ob = out_pool.tile([P, OW], bf16, tag="ob")
                nc.vector.tensor_copy(ob[:], pso[:, :OW])
                nc.sync.dma_start(out_v[:, mt, n * OW : (n + 1) * OW], ob[:])

        def out_proj_chunk_wide(mt):
            # phase 4 only: scores psum pool is idle; one [P, 2*OW] tile
            # covers both n-chunks -> one copy + one store per mt row.
            pso = ps_sc.tile([P, 1024], f32, tag="sc")
            for n in range(NO):
                for jo in range(MQ):
                    nc.tensor.matmul(
                        pso[:, n * OW : n * OW + OW],
                        outT[:, jo, mt * P : (mt + 1) * P],
                        wo_sb[:, jo, n * OW : (n + 1) * OW],
                        start=(jo == 0),
                        stop=(jo == MQ - 1),
                    )
            ob = out_pool.tile([P, 2 * OW], bf16, tag="obw")
            nc.vector.tensor_copy(ob[:], pso[:])
            nc.sync.dma_start(out_v[:, mt, :], ob[:])

        # ---- Phase 2: per head-pair KV proj + first-half attention ----
        for j in range(NP):
            vgt_sb = kv_proj(j)
            if j + 2 < NP:
                issue_kv_dma(j + 2)
            attention(j, 0, vgt_sb)

        # ---- Phase 3: second-half attention interleaved with out-proj(0) ----
        for j in range(NP):
            attention(j, 1)
            for mt in range(2 * j, 2 * j + 2):
                out_proj_chunk(mt)

        # ---- Phase 4: out-proj for the second tq half ----
        for mt in range(MT_PER_T2, 2 * MT_PER_T2):
            out_proj_chunk_wide(mt)


_PROG = None


def _get_program():
    global _PROG
    if _PROG is None:
        _PROG = build_program(D=D_F, TQ=TQ_F, L=L_F, NH=NH_LOC, num_devices=NCORES)
    return _PROG


def make_core_inputs(q, k, v, Wq, bq, Wk, bk, Wv, bv, Wo, bo):
    """Shard the full inputs into the 8 per-core input maps."""
    D, L, S, NH = D_F, L_F, STRIDE, NH_LOC
    QD = NH * DH
    MQ = QD // P

    q = np.ascontiguousarray(np.asarray(q, np.float32))
    k = np.ascontiguousarray(np.asarray(k, np.float32))
    v = np.ascontiguousarray(np.asarray(v, np.float32))

    qT = [np.ascontiguousarray(q[b].T).astype(BF16) for b in range(B)]
    # k[b] reshaped [L, S, D]; head h uses rows h::16 -> [:, h, :]
    # one [D, 16, L] permute per batch, shared by the two head-group cores
    kgt_full = [
        np.ascontiguousarray(k[b].reshape(L, S, D).transpose(2, 1, 0)).astype(BF16)
        for b in range(B)
    ]
    vgt_full = [
        np.ascontiguousarray(v[b].reshape(L, S, D).transpose(2, 1, 0)).astype(BF16)
        for b in range(B)
    ]

    WqT = np.ascontiguousarray(np.asarray(Wq, np.float32).T).astype(BF16)  # [D, D]
    WkT = np.ascontiguousarray(np.asarray(Wk, np.float32).T).astype(BF16)
    WvT = np.ascontiguousarray(np.asarray(Wv, np.float32).T).astype(BF16)
    WoT = np.ascontiguousarray(np.asarray(Wo, np.float32).T).astype(BF16)  # [D, D]
    bq = np.asarray(bq, np.float32)
    bk = np.asarray(bk, np.float32)
    bv = np.asarray(bv, np.float32)

    in_maps = []
    for c in range(NCORES):
        b, g = divmod(c, 2)
        gsl = slice(g * QD, (g + 1) * QD)
        hs0 = g * NH
        kgt = np.ascontiguousarray(kgt_full[b][:, hs0 : hs0 + NH, :])
        vgt = np.ascontiguousarray(vgt_full[b][:, hs0 : hs0 + NH, :])
        in_maps.append(
            {
                "qt": qT[b],
                "kgt": kgt,
                "vgt": vgt,
                "wqt": np.ascontiguousarray(WqT[:, gsl]),
                "wkt": np.ascontiguousarray(WkT[:, gsl]),
                "wvt": np.ascontiguousarray(WvT[:, gsl]),
                "wot": np.ascontiguousarray(WoT[gsl, :]),
                "bq": np.ascontiguousarray(bq[gsl].reshape(MQ, P).T),
                "bk": np.ascontiguousarray(bk[gsl].reshape(MQ, P).T),
                "bv": np.ascontiguousarray(bv[gsl].reshape(MQ, P).T),
            }
        )
    return in_maps


def _bf16_to_f32(a):
    return (
        a.view(np.uint16).astype(np.uint32) << np.uint32(16)
    ).view(np.float32)


def combine_outputs(results, bo):
    bo = np.asarray(bo, np.float32)
    out = np.empty((B, TQ_F, D_F), np.float32)
    for b in range(B):
        out[b] = _bf16_to_f32(results[2 * b]["out"])
        out[b] += _bf16_to_f32(results[2 * b + 1]["out"])
        out[b] += bo
    return out


def kernel(q, k, v, Wq, bq, Wk, bk, Wv, bv, Wo, bo):
    from concourse.bass_utils import run_bass_kernel_spmd

    nc = _get_program()
    in_maps = make_core_inputs(q, k, v, Wq, bq, Wk, bk, Wv, bv, Wo, bo)
    res = run_bass_kernel_spmd(nc, in_maps, core_ids=list(range(NCORES)))
    return combine_outputs(res.results, bo)
